# revision 48
# baseline (speedup 1.0000x reference)
"""Batch MMD loss on 8 Trainium2 NeuronCores.

Reference math per batch (X, Y: [1024, 128]):
    Z = concat(X, Y)                       # [2048, 128]
    D2_ij = |z_i - z_j|^2
    bw = sum(D2) / (n^2 - n)               # detached bandwidth heuristic
    K = exp(-D2 / bw)
    loss_b = mean(K_XX) - 2 mean(K_XY) + mean(K_YY)
output = sum_b loss_b  (32 batches)

Kernel factorization (per batch; each core handles 4 batches):
    u = 1/bw = (n^2-n) / (2*(n*S - |s|^2)),  S = sum_i |z_i|^2, s = sum_i z_i
    K_ij = a_i a_j exp(2u <z_i,z_j>),        a_i = exp(-u |z_i|^2)
    loss_b = (1/N^2) v^T E v,  v_i = sign_i a_i,  E = exp(2u Z Z^T)

E is symmetric: only a circulant half of the 16x16 grid of 128x128 tiles is
computed (row-tile it owns the 8 tiles at circular distance 1..8, 7 for
it>=8, counted twice) plus the 16 diagonal tiles at weight 1. The 136
[128,128] Gram chunks are streamed through two [128,1536] PSUM tiles: PE
matmuls (bf16) fill a tile, one ACT exp converts it to bf16 E in SBUF, PE
[128,1]-column matmuls reduce it into a packed per-column-tile accumulator
r1c (one PSUM bank). Per batch, a Pool-engine multiply+reduce collapses
r1c into one column of qs_all; a single PE matmul + reduce at the very end
produces the core scalar. The host sums the 8 core scalars.

Scheduling notes (all engines execute their queues in order):
 - s = sum_i z_i comes from PE matmuls over the row-major zb tiles, so the
   bandwidth u never waits on the transposes.
 - zb pool has bufs=1: batch b+1's loads carry a WAR dependency on batch
   b's transposes/stats, which keeps the (exclusive) DMA-engine FIFO in
   the right order without explicit cross-engine sync.
 - batch b+1's PE stat matmuls are injected into the middle of batch b's
   Gram/reduce stream so they neither block early Grams nor miss the
   exp deadline.
 - the per-batch collapse runs on the Pool engine so the PE/ACT/DVE
   streams never wait on it.

Data parallel: batch dim 32 -> 4 per core across 8 cores.
"""

from contextlib import ExitStack

import numpy as np

import bass_rust
import concourse.bass as bass
import concourse.tile as tile
from concourse import mybir
from concourse.bass_utils import run_bass_kernel_spmd

FP32 = mybir.dt.float32
BF16 = mybir.dt.bfloat16
AF = mybir.ActivationFunctionType
ALU = mybir.AluOpType

B, N, D = 32, 1024, 128
NCORES = 8
BPC = B // NCORES          # batches per core
n2 = 2 * N                 # 2048 rows in Z
NT = n2 // 128             # 16 row tiles
TS = 128                   # tile size
PT = 1536                  # psum streaming tile width (3 banks)
CH = PT // TS              # 128-col chunks per psum tile
INV_N2 = 1.0 / (N * N)     # 2^-20, exact in fp32


def _split_multi_waits(nc):
    """The walrus build in this container allows a single sync-wait per
    instruction ("Too many sync wait commands"), while Tile freely attaches
    several. Hoist all but one wait of each instruction onto single-wait
    no-ops inserted just before it on the same engine."""
    nid = [0]
    for f in nc.m.functions:
        for bb in f.blocks:
            insts = bb.instructions
            out = []
            changed = False
            for i in insts:
                si = getattr(i, "sync_info", None)
                if si is not None and len(si.on_wait) > 1:
                    waits = list(si.on_wait)
                    for w in waits[:-1]:
                        nid[0] += 1
                        nop = mybir.InstNoOp(
                            name=f"I-waitsplit-{nid[0]}", ins=[], outs=[]
                        )
                        nop.engine = i.engine
                        nop.sync_info = bass_rust.SyncInfo(
                            on_wait=[w], on_update=[]
                        )
                        out.append(nop)
                    si.on_wait = [waits[-1]]
                    changed = True
                out.append(i)
            if changed:
                bb.instructions = out


def _owned(it):
    """(j0, width) of the circulant off-diagonal strip owned by row-tile it."""
    k = 8 if it < 8 else 7
    return (TS * (it + 1)) % n2, k * TS


def _column_plan():
    """Stream of 128-col Gram chunks packed into [128, PT] psum tiles.

    Returns a list of psum-tile plans; each is a list of chunks
    (it, jt, diag) in dst order (chunk c sits at dst cols c*TS..).
    """
    events = []
    for it in range(NT):
        j0, w = _owned(it)
        for c in range(w // TS):
            jt = ((j0 + c * TS) % n2) // TS
            events.append((it, jt, False))
    for jt in range(NT):
        events.append((jt, jt, True))
    return [events[i : i + CH] for i in range(0, len(events), CH)]


def _matmul_segs(chunks):
    """Merge consecutive chunks of one psum tile into matmul segments
    (it, src_col, dst_col, width): same row-tile, contiguous source
    columns, and the destination must stay inside one 512-col PSUM bank."""
    segs = []
    cur = None
    for idx, (it, jt, diag) in enumerate(chunks):
        dst = idx * TS
        src = jt * TS
        if (
            cur is not None
            and cur[0] == it
            and cur[1] + cur[3] == src
            and cur[2] // 512 == (cur[2] + cur[3] + TS - 1) // 512
        ):
            cur = (cur[0], cur[1], cur[2], cur[3] + TS)
        else:
            if cur is not None:
                segs.append(cur)
            cur = (it, src, dst, TS)
    if cur is not None:
        segs.append(cur)
    return segs


_PLAN = _column_plan()
_NCHUNK = sum(len(t) for t in _PLAN)  # 136


def build(debug=False):
    nc = bass.Bass(num_swdge_queues=4)
    x = nc.dram_tensor("x", [BPC, N, D], FP32, kind="ExternalInput")
    y = nc.dram_tensor("y", [BPC, N, D], FP32, kind="ExternalInput")
    # per-(partition, batch) partial sums; the host finishes the reduction
    out = nc.dram_tensor("out", [128, BPC], FP32, kind="ExternalOutput")

    with tile.TileContext(nc) as tc, ExitStack() as ctx:
        consts = ctx.enter_context(tc.tile_pool(name="consts", bufs=1))
        zb_p = ctx.enter_context(tc.tile_pool(name="zb", bufs=1))
        zt_p = ctx.enter_context(tc.tile_pool(name="zt", bufs=3))
        zsq_p = ctx.enter_context(tc.tile_pool(name="zsq", bufs=2))
        sm_p = ctx.enter_context(tc.tile_pool(name="sm", bufs=32))
        e_p = ctx.enter_context(tc.tile_pool(name="e", bufs=8))
        acc_p = ctx.enter_context(tc.tile_pool(name="acc", bufs=1))
        pbig = ctx.enter_context(tc.tile_pool(name="pbig", bufs=2, space="PSUM"))
        pr1 = ctx.enter_context(tc.tile_pool(name="pr1", bufs=1, space="PSUM"))
        psm = ctx.enter_context(tc.tile_pool(name="psm", bufs=1, space="PSUM"))

        # --- constants (once per core) ---
        ones_col = consts.tile([128, 1], FP32)
        nc.gpsimd.memset(ones_col, 1.0)
        ones_col_bf = consts.tile([128, 1], BF16)
        nc.gpsimd.memset(ones_col_bf, 1.0)
        ones_row = consts.tile([1, 128], FP32)
        nc.gpsimd.memset(ones_row, 1.0)
        zrow_bf = consts.tile([1, 128], BF16)
        nc.gpsimd.memset(zrow_bf, 0.0)
        ones16_bf = consts.tile([1, 16], BF16)
        nc.gpsimd.memset(ones16_bf, 1.0)
        # sign rows: +/-2 (off-diag strips, counted twice), +/-1 (diag),
        # +/-1/N^2 (final collapse); X tiles t<8 positive, Y tiles negative
        sgn2_16 = consts.tile([128, NT], FP32)
        nc.gpsimd.memset(sgn2_16[:, 0:8], 2.0)
        nc.gpsimd.memset(sgn2_16[:, 8:16], -2.0)
        sgn1_16 = consts.tile([128, NT], FP32)
        nc.gpsimd.memset(sgn1_16[:, 0:8], 1.0)
        nc.gpsimd.memset(sgn1_16[:, 8:16], -1.0)
        sgnv_16 = consts.tile([128, NT], FP32)
        nc.gpsimd.memset(sgnv_16[:, 0:8], INV_N2)
        nc.gpsimd.memset(sgnv_16[:, 8:16], -INV_N2)
        n2_col = consts.tile([128, 1], FP32)
        nc.gpsimd.memset(n2_col, float(n2))
        k2_row = consts.tile([1, 128], FP32)
        nc.gpsimd.memset(k2_row, float(n2 * n2 - n2))
        neghalf_col = consts.tile([128, 1], FP32)
        nc.gpsimd.memset(neghalf_col, -0.5)

        # preload the Exp activation table while the first loads run, so the
        # first real exp doesn't pay the ~1.3us table switch
        warm = consts.tile([1, 1], FP32)
        nc.scalar.activation(warm, ones_row[:, 0:1], AF.Exp, bias=0.0, scale=1.0)

        qs_all = acc_p.tile([128, BPC], FP32)
        st = [dict() for _ in range(BPC)]

        def emit_loads(b):
            # batch 0 is latency-critical: 4-tile granules pipeline the
            # (exclusive) DMA-engine FIFO; later batches use 8-tile chunks
            # to keep trigger overhead off the steady-state queues
            gran = 8
            zb = zb_p.tile([128, NT, D], BF16, tag="zb")
            for half, src in ((0, x), (1, y)):
                s_ap = src.ap()[b].rearrange("(t p) d -> p t d", p=128)
                for c in range(8 // gran):
                    g = half * 8 + c * gran
                    nc.gpsimd.dma_start(
                        out=zb[:, g : g + gran, :],
                        in_=s_ap[:, c * gran : (c + 1) * gran, :],
                    )
            zt = zt_p.tile([128, NT, D], BF16, tag="zt")
            for q in range(NT // gran):
                sl = slice(q * gran, (q + 1) * gran)
                nc.sync.dma_start(out=zt[:, sl, :], in_=zb[:, sl, :], transpose=True)
            st[b]["zb"] = zb
            st[b]["zt"] = zt

        def emit_stats_dve(b):
            """zsq = zb^2 with fused per-partition totals (feeds S for the
            bandwidth). The per-row sq_col reduction is deferred to
            emit_sq() since only S gates the first exp."""
            zb = st[b]["zb"]
            zsq = zsq_p.tile([128, NT, D], FP32, tag="zsq")
            for h in range(2):
                sl = slice(h * 8, (h + 1) * 8)
                nc.vector.tensor_tensor(
                    zsq[:, sl, :].rearrange("p t d -> p (t d)"),
                    zb[:, sl, :].rearrange("p t d -> p (t d)"),
                    zb[:, sl, :].rearrange("p t d -> p (t d)"),
                    ALU.mult,
                )
            st[b]["zsq"] = zsq

        def emit_sq(b):
            """Per-row |z_i|^2 in the [p, t] layout (for the a_i weights).
            Quarter-granularity keeps each op short, so the list scheduler
            can slot the (late-ready, urgent) u-chain ops between them."""
            zsq = st[b]["zsq"]
            sq_col = sm_p.tile([128, NT], FP32, tag="sqcol")
            for q in range(4):
                sl = slice(q * 4, (q + 1) * 4)
                nc.vector.tensor_reduce(
                    out=sq_col[:, sl], in_=zsq[:, sl, :],
                    axis=mybir.AxisListType.X, op=ALU.add,
                )
            st[b]["sq_col"] = sq_col

        def emit_stats_pe1(b):
            """s = sum_i z_i via PE over the row-major zb tiles. All small
            ops go to the Pool engine: DVE is busy with the zsq reduces and
            its in-order queue would sit on the critical path."""
            zb = st[b]["zb"]
            s_ps = psm.tile([128, 1], FP32, tag="psm")
            for t in range(NT):
                nc.tensor.matmul(
                    s_ps, lhsT=zb[:, t, :], rhs=ones_col_bf,
                    start=(t == 0), stop=(t == NT - 1),
                )
            # PSUM reads must go via DVE (GPSIMD cannot access PSUM on HW);
            # the SBUF-only follow-ups go to Pool to stay off DVE's queue
            s_sb = sm_p.tile([128, 1], FP32, tag="ssb")
            nc.vector.tensor_copy(s_sb, s_ps)
            negs = sm_p.tile([128, 1], FP32, tag="negs")
            nc.gpsimd.tensor_scalar_mul(negs, s_sb, -1.0)
            st[b]["s_sb"] = s_sb
            st[b]["negs"] = negs

        def emit_stats_pe2(b):
            """2u = (n2^2-n2) / (n2*S - |s|^2). S comes from per-feature
            sums of zsq on PE, so no big DVE reduce sits on the u path."""
            zsq = st[b]["zsq"]
            F_ps = psm.tile([128, 1], FP32, tag="psm")
            for t in range(NT):
                nc.tensor.matmul(
                    F_ps, lhsT=zsq[:, t, :], rhs=ones_col,
                    start=(t == 0), stop=(t == NT - 1),
                )
            F_sb = sm_p.tile([128, 1], FP32, tag="Fsb")
            nc.vector.tensor_copy(F_sb, F_ps)
            diff_ps = psm.tile([1, 1], FP32, tag="psm")
            nc.tensor.matmul(
                diff_ps, lhsT=F_sb, rhs=n2_col,
                start=True, stop=False, skip_group_check=True,
            )
            nc.tensor.matmul(
                diff_ps, lhsT=st[b]["s_sb"], rhs=st[b]["negs"],
                start=False, stop=True, skip_group_check=True,
            )
            rec_sb = sm_p.tile([1, 1], FP32, tag="rec")
            nc.vector.reciprocal(rec_sb, diff_ps)
            u2_ps = psm.tile([128, 1], FP32, tag="psm")
            nc.tensor.matmul(u2_ps, lhsT=k2_row, rhs=rec_sb, start=True, stop=True)
            scale2u = sm_p.tile([128, 1], FP32, tag="scale2u")
            nc.vector.tensor_copy(scale2u, u2_ps)
            negu = sm_p.tile([128, 1], FP32, tag="negu")
            nc.gpsimd.tensor_tensor(negu, scale2u, neghalf_col, ALU.mult)
            st[b]["scale2u"] = scale2u
            st[b]["negu"] = negu

        def emit_weights(b):
            """a_i = exp(-u |z_i|^2) and the signed weight columns."""
            a_col = sm_p.tile([128, NT], FP32, tag="acol")
            nc.scalar.activation(
                a_col, st[b]["sq_col"], AF.Exp, bias=0.0, scale=st[b]["negu"]
            )
            av2_bf = sm_p.tile([128, NT], BF16, tag="av2bf")
            nc.vector.tensor_tensor(av2_bf, a_col, sgn2_16, ALU.mult)
            avd_bf = sm_p.tile([128, NT], BF16, tag="avdbf")
            nc.vector.tensor_tensor(avd_bf, a_col, sgn1_16, ALU.mult)
            avn_col = sm_p.tile([128, NT], FP32, tag="avncol")
            nc.vector.tensor_tensor(avn_col, a_col, sgnv_16, ALU.mult)
            st[b]["av2"] = av2_bf
            st[b]["avd"] = avd_bf
            st[b]["avn"] = avn_col

        def emit_stream(b, hooks):
            """Gram -> exp -> weighted reduction stream for batch b.
            hooks: {tile_index: fn} emission-order injections."""
            zt = st[b]["zt"]
            zt_f = zt[:, :, :].rearrange("p t d -> p (t d)")

            r1c_ps = None
            nred = 0
            pending = []

            def emit_reds(e_sb, chunks):
                nonlocal nred
                for c, (it, jt, diag) in enumerate(chunks):
                    nred += 1
                    wcol = st[b]["avd"] if diag else st[b]["av2"]
                    nc.tensor.matmul(
                        r1c_ps[:, jt : jt + 1],
                        lhsT=e_sb[:, c * TS : (c + 1) * TS],
                        rhs=wcol[:, it : it + 1],
                        start=False,
                        stop=(nred == _NCHUNK),
                        skip_group_check=True,
                    )

            for ti, chunks in enumerate(_PLAN):
                used = len(chunks) * TS
                if used <= 512:
                    # the short last tile fits the stats bank; keeping it out
                    # of pbig gives that pool an odd tile count per batch, so
                    # the next batch's first Gram lands on a long-freed slot
                    p_ps = psm.tile([128, 512], FP32, tag="psm")
                else:
                    p_ps = pbig.tile([128, PT], FP32, tag="bigP")
                for (it, src, dst, w) in _matmul_segs(chunks):
                    nc.tensor.matmul(
                        p_ps[:, dst : dst + w],
                        lhsT=zt[:, it, :],
                        rhs=zt_f[:, src : src + w],
                        start=True,
                        stop=True,
                    )
                if ti == 0:
                    # open the r1c accumulation group only now, so the WAR
                    # wait on batch b-1's collapse never delays tile-0 Grams
                    r1c_ps = pr1.tile([128, NT], FP32, tag="R1C")
                    nc.tensor.matmul(
                        r1c_ps, lhsT=zrow_bf, rhs=ones16_bf,
                        start=True, stop=False, skip_group_check=True,
                    )
                e_sb = e_p.tile([128, PT], BF16, tag="E")
                nc.scalar.activation(
                    e_sb[:, 0:used], p_ps[:, 0:used], AF.Exp,
                    bias=0.0, scale=st[b]["scale2u"],
                )
                # the hook sits between exp and reductions so batch-0's
                # weight hook (tile 0) is emitted before the first reduction
                # references the weight tiles, while its a_col lands after
                # exp_t0 in the ACT queue. Reductions trail by one tile so
                # the in-order PE queue never waits on exp_ti before issuing
                # tile ti+1's Grams.
                hk = hooks.get(ti)
                if hk is not None:
                    hk()
                pending.append((e_sb, chunks))
                if len(pending) > 1:
                    emit_reds(*pending.pop(0))
            while pending:
                emit_reds(*pending.pop(0))

            # per-batch collapse: qs_all[p, b] = sum_t r1c[p, t] * (sgn_t a/N^2)
            q16 = sm_p.tile([128, NT], FP32, tag="q16")
            nc.vector.tensor_tensor(q16, r1c_ps, st[b]["avn"], ALU.mult)
            nc.vector.tensor_reduce(
                out=qs_all[:, b : b + 1], in_=q16,
                axis=mybir.AxisListType.X, op=ALU.add,
            )

        # software pipeline (every engine queue is in-order):
        #   loads/stats of batch b+1 are emitted before batch b's stream;
        #   batch b+1's PE stat-matmuls (tile 6), u-chain + row-sq (tile 8)
        #   and weights (tile 10) are injected mid-stream so they neither
        #   block early Grams nor miss their deadlines. Batch 0's weights
        #   hook in at tile 1 of its own stream (its a_col must not sit
        #   ahead of the first exps in the ACT queue).
        for b in range(BPC):
            emit_loads(b)
            emit_stats_dve(b)
            if b == 0:
                emit_stats_pe1(0)
                emit_stats_pe2(0)
                emit_sq(0)
            else:
                hooks = {
                    6: lambda bb=b: emit_stats_pe1(bb),
                    8: lambda bb=b: (emit_stats_pe2(bb), emit_sq(bb)),
                    10: lambda bb=b: emit_weights(bb),
                }
                if b == 1:
                    hooks[0] = lambda: emit_weights(0)
                emit_stream(b - 1, hooks)
        emit_stream(BPC - 1, {})

        # ship the per-partition partials; the host sums 128*BPC floats
        nc.sync.dma_start(out=out.ap(), in_=qs_all)

    _split_multi_waits(nc)
    return nc


_CACHE = {}


def _get_nc():
    if "nc" not in _CACHE:
        _CACHE["nc"] = build()
    return _CACHE["nc"]


def kernel(allX: np.ndarray, allY: np.ndarray) -> np.ndarray:
    allX = np.ascontiguousarray(allX, dtype=np.float32)
    allY = np.ascontiguousarray(allY, dtype=np.float32)
    nc = _get_nc()
    in_maps = [
        {
            "x": allX[i * BPC : (i + 1) * BPC],
            "y": allY[i * BPC : (i + 1) * BPC],
        }
        for i in range(NCORES)
    ]
    res = run_bass_kernel_spmd(nc, in_maps, core_ids=list(range(NCORES)))
    total = np.float32(0.0)
    for r in res.results:
        total += np.float32(np.asarray(r["out"], dtype=np.float32).sum())
    return np.asarray(total, dtype=np.float32)


if __name__ == "__main__":
    rng = np.random.default_rng(0)
    ax = rng.standard_normal((B, N, D)).astype(np.float32)
    ay = rng.standard_normal((B, N, D)).astype(np.float32)
    print(kernel(ax, ay))
